# revision 1
# baseline (speedup 1.0000x reference)
"""BinarizedLinear Trainium2 kernel.

out = sign(x) @ sign(W).T + bias,  x:[8192,4096] W:[4096,4096] bias:[4096] (fp32)

Strategy:
  - Data-parallel over tokens: 8 cores x 1024 rows of x each.
  - Host passes x.T and W.T (pure layout; all math on device) so the
    contraction dim (IN) lands on SBUF partitions with natural DMAs.
  - On device per core: binarize x -> +-0.5 (one DVE tensor_scalar op:
    is_ge(x,0) -> {0,1}, subtract 0.5 -> +-0.5), cached in SBUF.
    Stream W.T in 512-wide column tiles, binarize the same way, and run
    K=4096 accumulation chains on the PE.  PSUM holds 0.25*S exactly
    (S = integer +-1 dot product), evicted as out = 4*psum + bias which
    is bit-exact vs the fp32 reference.
  - bias is passed host-replicated to [128, OUT] (layout only) so the
    free-axis bias add is a plain DVE tensor_tensor.
"""

import os
import sys

sys.path.insert(0, "/opt/trn_rl_repo")

import numpy as np

import concourse.bass as bass
import concourse.mybir as mybir
import concourse.tile as tile
from concourse import bacc
from concourse.bass import ts

N_CORES = 8
P = 128

# Full problem shapes (hardcoded per contract)
N_TOK, D_IN, D_OUT = 8192, 4096, 4096


def build_nc(
    t_loc: int = N_TOK // N_CORES,
    d_in: int = D_IN,
    d_out: int = D_OUT,
    n_tile: int = 512,
    mm_dtype: mybir.dt = mybir.dt.bfloat16,
    double_row: bool = False,
):
    """Build the per-core Bass program.

    Inputs (per core):
      xT   [d_in, t_loc] fp32   (x.T shard)
      wT   [d_in, d_out] fp32   (W.T, full)
      bias [128, d_out]  fp32   (host-replicated rows)
    Output:
      out  [t_loc, d_out] fp32
    """
    assert t_loc % P == 0 and d_in % P == 0 and d_out % n_tile == 0
    k_tiles = d_in // P
    m_tiles = t_loc // P
    n_tiles = d_out // n_tile
    if double_row:
        assert mm_dtype in (mybir.dt.float8e4, mybir.dt.float8e5)
        assert k_tiles % 2 == 0

    nc = bacc.Bacc("TRN2", target_bir_lowering=False, debug=False)

    xT = nc.dram_tensor("xT", [d_in, t_loc], mybir.dt.float32, kind="ExternalInput")
    wT = nc.dram_tensor("wT", [d_in, d_out], mybir.dt.float32, kind="ExternalInput")
    bias = nc.dram_tensor("bias", [P, d_out], mybir.dt.float32, kind="ExternalInput")
    out = nc.dram_tensor("out", [t_loc, d_out], mybir.dt.float32, kind="ExternalOutput")

    ge = mybir.AluOpType.is_ge
    sub = mybir.AluOpType.subtract
    add = mybir.AluOpType.add

    with tile.TileContext(nc) as tc:
        with (
            tc.tile_pool(name="const", bufs=1) as const_pool,
            tc.tile_pool(name="xbin_pool", bufs=1) as xbin_pool,
            tc.tile_pool(name="xstage", bufs=3) as xstage_pool,
            tc.tile_pool(name="wstage", bufs=6) as wstage_pool,
            tc.tile_pool(name="wbin", bufs=2) as wbin_pool,
            tc.tile_pool(name="evict", bufs=8) as evict_pool,
            tc.tile_pool(name="psum", bufs=6, space="PSUM") as psum_pool,
        ):
            # bias, replicated on host to [P, d_out]
            bias_sb = const_pool.tile([P, d_out], mybir.dt.float32, name="bias_sb")
            nc.sync.dma_start(bias_sb[:], bias[:])

            # ---- load + binarize all of x.T into SBUF (mm_dtype, +-0.5) ----
            xbin = xbin_pool.tile([P, k_tiles, t_loc], mm_dtype, name="xbin")
            for k in range(k_tiles):
                xf = xstage_pool.tile([P, t_loc], mybir.dt.float32, name="xf")
                nc.sync.dma_start(xf[:], xT[ts(k, P), :])
                nc.vector.tensor_scalar(xbin[:, k, :], xf[:], 0.0, 0.5, ge, sub)

            # ---- stream W.T by n-tile; matmul; evict ----
            for n in range(n_tiles):
                wbin = wbin_pool.tile([P, k_tiles, n_tile], mm_dtype, name="wbin")
                for k in range(k_tiles):
                    wf = wstage_pool.tile([P, n_tile], mybir.dt.float32, name="wf")
                    nc.sync.dma_start(wf[:], wT[ts(k, P), ts(n, n_tile)])
                    nc.vector.tensor_scalar(wbin[:, k, :], wf[:], 0.0, 0.5, ge, sub)

                for m in range(m_tiles):
                    psum = psum_pool.tile([P, n_tile], mybir.dt.float32, name="psum")
                    if double_row:
                        for k in range(0, k_tiles, 2):
                            nc.tensor.matmul(
                                psum[:],
                                xbin[:, k : k + 2, ts(m, P)],
                                wbin[:, k : k + 2, :],
                                start=(k == 0),
                                stop=(k == k_tiles - 2),
                                perf_mode=mybir.MatmulPerfMode.DoubleRow,
                            )
                    else:
                        for k in range(k_tiles):
                            nc.tensor.matmul(
                                psum[:],
                                xbin[:, k, ts(m, P)],
                                wbin[:, k, :],
                                start=(k == 0),
                                stop=(k == k_tiles - 1),
                            )
                    # out = 4 * psum + bias   (exact: psum = S/4, S integer)
                    t1 = evict_pool.tile([P, n_tile], mybir.dt.float32, name="t1")
                    nc.scalar.activation(
                        t1[:], psum[:], mybir.ActivationFunctionType.Copy, scale=4.0
                    )
                    ob = evict_pool.tile([P, n_tile], mybir.dt.float32, name="ob")
                    nc.vector.tensor_tensor(
                        ob[:], t1[:], bias_sb[:, ts(n, n_tile)], add
                    )
                    nc.sync.dma_start(out[ts(m, P), ts(n, n_tile)], ob[:])

    nc.compile()
    return nc


_NC_CACHE: dict = {}


def _get_nc(key=("full",)):
    if key not in _NC_CACHE:
        _NC_CACHE[key] = build_nc()
    return _NC_CACHE[key]


def kernel(x: np.ndarray, fp_weight: np.ndarray, fp_bias: np.ndarray) -> np.ndarray:
    assert x.shape == (N_TOK, D_IN) and fp_weight.shape == (D_OUT, D_IN)
    from concourse.bass_utils import run_bass_kernel_spmd

    nc = _get_nc()

    xT = np.ascontiguousarray(np.asarray(x, dtype=np.float32).T)  # [D_IN, N_TOK]
    wT = np.ascontiguousarray(np.asarray(fp_weight, dtype=np.float32).T)
    bias_rep = np.ascontiguousarray(
        np.broadcast_to(np.asarray(fp_bias, dtype=np.float32)[None, :], (P, D_OUT))
    )

    t_loc = N_TOK // N_CORES
    in_maps = [
        {
            "xT": np.ascontiguousarray(xT[:, i * t_loc : (i + 1) * t_loc]),
            "wT": wT,
            "bias": bias_rep,
        }
        for i in range(N_CORES)
    ]

    res = run_bass_kernel_spmd(nc, in_maps, core_ids=list(range(N_CORES)))
    return np.concatenate([res.results[i]["out"] for i in range(N_CORES)], axis=0)


# revision 22
# speedup vs baseline: 11.2584x; 11.2584x over previous
"""BinarizedLinear Trainium2 kernel.

out = sign(x) @ sign(W).T + bias,  x:[8192,4096] W:[4096,4096] bias:[4096] (fp32)

Strategy:
  - Data-parallel over tokens: 8 cores x 1024 rows of x each.
  - Host passes x.T and W.T (pure layout; all math on device) so the
    contraction dim (IN) lands on SBUF partitions with natural DMAs.
  - On device per core: binarize x -> +-0.5 (one DVE tensor_scalar op:
    is_ge(x,0) -> {0,1}, subtract 0.5 -> +-0.5), cached in SBUF.
    Stream W.T in 512-wide column tiles, binarize the same way, and run
    K=4096 accumulation chains on the PE.  PSUM holds 0.25*S exactly
    (S = integer +-1 dot product), evicted as out = 4*psum + bias which
    is bit-exact vs the fp32 reference.
  - bias is passed host-replicated to [128, OUT] (layout only) so the
    free-axis bias add is a plain DVE tensor_tensor.
"""

import os
import sys

sys.path.insert(0, "/opt/trn_rl_repo")

import numpy as np

import concourse.bass as bass
import concourse.mybir as mybir
import concourse.tile as tile
from concourse import bacc
from concourse.bass import ts

N_CORES = 8
P = 128

# Full problem shapes (hardcoded per contract)
N_TOK, D_IN, D_OUT = 8192, 4096, 4096


def build_nc(
    t_loc: int = N_TOK // N_CORES,
    d_in: int = D_IN,
    d_out: int = D_OUT,
    n_tile: int = 512,
    mm_dtype: mybir.dt = mybir.dt.bfloat16,
    double_row: bool = False,
    repeat: int = 1,
    n_pair: int = 1,
):
    """Build the per-core Bass program.

    Inputs (per core):
      xT   [d_in, t_loc] fp32   (x.T shard)
      wT   [d_in, d_out] fp32   (W.T, full)
      bias [128, d_out]  fp32   (host-replicated rows)
    Output:
      out  [t_loc, d_out] fp32
    """
    assert t_loc % P == 0 and d_in % P == 0 and d_out % n_tile == 0
    k_tiles = d_in // P
    m_tiles = t_loc // P
    n_tiles = d_out // n_tile
    if double_row:
        assert mm_dtype in (mybir.dt.float8e4, mybir.dt.float8e5)
        assert k_tiles % 2 == 0

    nc = bacc.Bacc("TRN2", target_bir_lowering=False, debug=False)

    xT = nc.dram_tensor("xT", [d_in, t_loc], mybir.dt.float32, kind="ExternalInput")
    wT = nc.dram_tensor("wT", [d_in, d_out], mybir.dt.float32, kind="ExternalInput")
    bias = nc.dram_tensor("bias", [P, d_out], mybir.dt.float32, kind="ExternalInput")
    out = nc.dram_tensor("out", [t_loc, d_out], mybir.dt.float32, kind="ExternalOutput")

    ge = mybir.AluOpType.is_ge
    sub = mybir.AluOpType.subtract
    add = mybir.AluOpType.add

    with tile.TileContext(nc) as tc:
        with (
            tc.tile_pool(name="const", bufs=1) as const_pool,
            tc.tile_pool(name="xbin_pool", bufs=1) as xbin_pool,
            tc.tile_pool(name="xstage", bufs=3) as xstage_pool,
            tc.tile_pool(name="wstage", bufs=6) as wstage_pool,
            tc.tile_pool(name="wbin", bufs=2 * n_pair) as wbin_pool,
            tc.tile_pool(name="evict", bufs=8) as evict_pool,
            tc.tile_pool(name="psum", bufs=6, space="PSUM") as psum_pool,
        ):
            # bias, replicated on host to [P, d_out]
            bias_sb = const_pool.tile([P, d_out], mybir.dt.float32, name="bias_sb")
            nc.sync.dma_start(bias_sb[:], bias[:])

            import contextlib

            rep_ctx = (
                tc.For_i(0, repeat, 1) if repeat > 1 else contextlib.nullcontext()
            )
            with rep_ctx:
                _body(
                    nc, tc, xT, wT, out, bias_sb,
                    xbin_pool, xstage_pool, wstage_pool, wbin_pool, evict_pool,
                    psum_pool, t_loc, n_tile, k_tiles, m_tiles, n_tiles,
                    mm_dtype, double_row, n_pair,
                )

    nc.compile()
    return nc


def _body(
    nc, tc, xT, wT, out, bias_sb,
    xbin_pool, xstage_pool, wstage_pool, wbin_pool, evict_pool,
    psum_pool, t_loc, n_tile, k_tiles, m_tiles, n_tiles,
    mm_dtype, double_row, n_pair=1,
):
    ge = mybir.AluOpType.is_ge
    sub = mybir.AluOpType.subtract
    add = mybir.AluOpType.add
    assert n_tiles % n_pair == 0

    # ---- load + binarize all of x.T into SBUF (mm_dtype, +-0.5) ----
    xbin = xbin_pool.tile([P, k_tiles, t_loc], mm_dtype, name="xbin")
    for k in range(k_tiles):
        xf = xstage_pool.tile([P, t_loc], mybir.dt.float32, name="xf")
        nc.sync.dma_start(xf[:], xT[ts(k, P), :])
        nc.vector.tensor_scalar(xbin[:, k, :], xf[:], 0.0, 0.5, ge, sub)

    # ---- stream W.T by group of n_pair n-tiles; matmul; evict ----
    for ng in range(n_tiles // n_pair):
        wbins = []
        for j in range(n_pair):
            n = ng * n_pair + j
            wbin = wbin_pool.tile([P, k_tiles, n_tile], mm_dtype, name="wbin")
            for k in range(k_tiles):
                wf = wstage_pool.tile([P, n_tile], mybir.dt.float32, name="wf")
                nc.sync.dma_start(wf[:], wT[ts(k, P), ts(n, n_tile)])
                nc.vector.tensor_scalar(wbin[:, k, :], wf[:], 0.0, 0.5, ge, sub)
            wbins.append(wbin)

        for m in range(m_tiles):
            psums = [
                psum_pool.tile([P, n_tile], mybir.dt.float32, name="psum")
                for _ in range(n_pair)
            ]
            if double_row:
                for k in range(0, k_tiles, 2):
                    for j in range(n_pair):
                        nc.tensor.matmul(
                            psums[j][:],
                            xbin[:, k : k + 2, ts(m, P)],
                            wbins[j][:, k : k + 2, :],
                            start=(k == 0),
                            stop=(k == k_tiles - 2),
                            perf_mode=mybir.MatmulPerfMode.DoubleRow,
                        )
            else:
                for k in range(k_tiles):
                    for j in range(n_pair):
                        nc.tensor.matmul(
                            psums[j][:],
                            xbin[:, k, ts(m, P)],
                            wbins[j][:, k, :],
                            start=(k == 0),
                            stop=(k == k_tiles - 1),
                        )
            for j in range(n_pair):
                n = ng * n_pair + j
                # out = 4 * psum + bias   (exact: psum = S/4, S integer)
                t1 = evict_pool.tile([P, n_tile], mybir.dt.float32, name="t1")
                nc.scalar.activation(
                    t1[:], psums[j][:], mybir.ActivationFunctionType.Copy, scale=4.0
                )
                ob = evict_pool.tile([P, n_tile], mybir.dt.float32, name="ob")
                nc.vector.tensor_tensor(
                    ob[:], t1[:], bias_sb[:, ts(n, n_tile)], add
                )
                nc.sync.dma_start(out[ts(m, P), ts(n, n_tile)], ob[:])


def build_nc_v3(
    t_loc: int = 2048,
    d_in: int = D_IN,
    d_out_loc: int = 2048,
    n_tile: int = 512,
    mm_dtype: mybir.dt = mybir.dt.float8e4,
    double_row: bool = True,
    repeat: int = 1,
    mb_size: int = 4,
    kb_tiles: int = 8,
    x_m_major: bool = False,
    w_sign_act: bool = False,
):
    """4x2-sharded variant: tokens split 4 ways, out-features 2 ways.

    Per core: xT [d_in, t_loc], wT [d_in, d_out_loc], bias [P, d_out_loc],
    out [t_loc, d_out_loc].  x binarized+cached in SBUF; W streamed by
    n-tile.  Matmuls are emitted kb-block-major so the in-order PE stream
    tracks DMA arrival order during the ramp.
    """
    assert t_loc % P == 0 and d_in % P == 0 and d_out_loc % n_tile == 0
    k_tiles = d_in // P
    m_tiles = t_loc // P
    n_tiles = d_out_loc // n_tile
    assert m_tiles % mb_size == 0 and k_tiles % kb_tiles == 0
    if double_row:
        assert kb_tiles % 2 == 0

    nc = bacc.Bacc("TRN2", target_bir_lowering=False, debug=False)
    xT = nc.dram_tensor("xT", [d_in, t_loc], mybir.dt.float32, kind="ExternalInput")
    wT = nc.dram_tensor("wT", [d_in, d_out_loc], mybir.dt.float32, kind="ExternalInput")
    bias = nc.dram_tensor("bias", [P, d_out_loc], mybir.dt.float32, kind="ExternalInput")
    out = nc.dram_tensor(
        "out", [t_loc, d_out_loc], mybir.dt.float32, kind="ExternalOutput"
    )

    ge = mybir.AluOpType.is_ge
    sub = mybir.AluOpType.subtract
    add = mybir.AluOpType.add
    k_step = 2 if double_row else 1
    perf = mybir.MatmulPerfMode.DoubleRow if double_row else None
    kb_blocks = k_tiles // kb_tiles

    with tile.TileContext(nc) as tc:
        with (
            tc.tile_pool(name="const", bufs=1) as const_pool,
            tc.tile_pool(name="xbin_pool", bufs=1) as xbin_pool,
            tc.tile_pool(name="xstage", bufs=3) as xstage_pool,
            tc.tile_pool(name="wstage", bufs=6) as wstage_pool,
            tc.tile_pool(name="wbin", bufs=2) as wbin_pool,
            tc.tile_pool(name="evict", bufs=8) as evict_pool,
            tc.tile_pool(name="psum", bufs=8, space="PSUM") as psum_pool,
        ):
            bias_sb = const_pool.tile([P, d_out_loc], mybir.dt.float32, name="bias_sb")
            nc.sync.dma_start(bias_sb[:], bias[:])

            import contextlib

            rep_ctx = (
                tc.For_i(0, repeat, 1) if repeat > 1 else contextlib.nullcontext()
            )
            with rep_ctx:
                xbin = xbin_pool.tile([P, k_tiles, t_loc], mm_dtype, name="xbin")
                if x_m_major:
                    # Load x by token-block (all k per block) so the first
                    # matmul group's operands arrive early — matches the
                    # (mb, kb) consumption order of the in-order PE stream.
                    mblk = mb_size * P
                    for mb0 in range(t_loc // mblk):
                        for k in range(k_tiles):
                            xf = xstage_pool.tile(
                                [P, mblk], mybir.dt.float32, name="xf"
                            )
                            nc.sync.dma_start(
                                xf[:], xT[ts(k, P), ts(mb0, mblk)]
                            )
                            nc.vector.tensor_scalar(
                                xbin[:, k, ts(mb0, mblk)], xf[:], 0.0, 0.5, ge, sub
                            )
                else:
                    for k in range(k_tiles):
                        xf = xstage_pool.tile(
                            [P, t_loc], mybir.dt.float32, name="xf"
                        )
                        nc.sync.dma_start(xf[:], xT[ts(k, P), :])
                        nc.vector.tensor_scalar(
                            xbin[:, k, :], xf[:], 0.0, 0.5, ge, sub
                        )

                for n in range(n_tiles):
                    wbin = wbin_pool.tile([P, k_tiles, n_tile], mm_dtype, name="wbin")
                    for k in range(k_tiles):
                        wf = wstage_pool.tile(
                            [P, n_tile], mybir.dt.float32, name="wf"
                        )
                        nc.sync.dma_start(wf[:], wT[ts(k, P), ts(n, n_tile)])
                        if w_sign_act:
                            nc.scalar.sign(wbin[:, k, :], wf[:])
                        else:
                            nc.vector.tensor_scalar(
                                wbin[:, k, :], wf[:], 0.0, 0.5, ge, sub
                            )

                    for mb in range(m_tiles // mb_size):
                        psums = [
                            psum_pool.tile(
                                [P, n_tile], mybir.dt.float32, name="psum"
                            )
                            for _ in range(mb_size)
                        ]
                        for kb in range(kb_blocks):
                            for mi in range(mb_size):
                                m = mb * mb_size + mi
                                for kp in range(0, kb_tiles, k_step):
                                    k = kb * kb_tiles + kp
                                    nc.tensor.matmul(
                                        psums[mi][:],
                                        xbin[:, k : k + k_step, ts(m, P)],
                                        wbin[:, k : k + k_step, :],
                                        start=(kb == 0 and kp == 0),
                                        stop=(
                                            kb == kb_blocks - 1
                                            and kp == kb_tiles - k_step
                                        ),
                                        perf_mode=perf,
                                    )
                        for mi in range(mb_size):
                            m = mb * mb_size + mi
                            t1 = evict_pool.tile(
                                [P, n_tile], mybir.dt.float32, name="t1"
                            )
                            if w_sign_act:
                                # ACT is busy with W Sign; scale on DVE
                                nc.vector.tensor_scalar_mul(
                                    t1[:], psums[mi][:], 2.0
                                )
                            else:
                                nc.scalar.activation(
                                    t1[:],
                                    psums[mi][:],
                                    mybir.ActivationFunctionType.Copy,
                                    scale=4.0,
                                )
                            ob = evict_pool.tile(
                                [P, n_tile], mybir.dt.float32, name="ob"
                            )
                            nc.vector.tensor_tensor(
                                ob[:], t1[:], bias_sb[:, ts(n, n_tile)], add
                            )
                            nc.sync.dma_start(out[ts(m, P), ts(n, n_tile)], ob[:])

    nc.compile()
    return nc


def build_nc_v4(
    t_loc: int = 2048,
    d_in: int = D_IN,
    d_out_loc: int = 2048,
    o_group: int = 512,
    t_tile: int = 512,
    mm_dtype: mybir.dt = mybir.dt.float8e4,
    double_row: bool = True,
    repeat: int = 1,
    kb_tiles: int = 8,
    diag: str = "full",  # "full" | "no_w" | "no_x" | "no_bin" (timing diagnostics)
):
    """out.T orientation: W chunks are the stationary operand, x moving.

    Each LDWEIGHTS (wbin [128, 2, 128]) serves t_loc/t_tile consecutive
    matmuls (moving over token tiles), hiding the DoubleRow weight-load
    cost.  PSUM is [outf, tokens]; eviction is a single DVE
    tensor_scalar(psum*4 + bias[p]) with per-partition bias AP; output is
    written as out.T [d_out_loc, t_loc] and un-transposed on host.

    Inputs per core: xT [d_in, t_loc], wT [d_in, d_out_loc],
    biasT [P, d_out_loc//P] (bias.reshape(-1, P).T), outT [d_out_loc, t_loc].
    """
    assert t_loc % t_tile == 0 and d_in % P == 0 and d_out_loc % o_group == 0
    k_tiles = d_in // P
    o_blocks = d_out_loc // P
    o_per_g = o_group // P
    t_blocks = t_loc // t_tile
    kb_blocks = k_tiles // kb_tiles
    if double_row:
        assert kb_tiles % 2 == 0
    k_step = 2 if double_row else 1
    perf = mybir.MatmulPerfMode.DoubleRow if double_row else None

    nc = bacc.Bacc("TRN2", target_bir_lowering=False, debug=False)
    xT = nc.dram_tensor("xT", [d_in, t_loc], mybir.dt.float32, kind="ExternalInput")
    wT = nc.dram_tensor("wT", [d_in, d_out_loc], mybir.dt.float32, kind="ExternalInput")
    biasT = nc.dram_tensor(
        "biasT", [P, o_blocks], mybir.dt.float32, kind="ExternalInput"
    )
    outT = nc.dram_tensor(
        "outT", [d_out_loc, t_loc], mybir.dt.float32, kind="ExternalOutput"
    )

    ge = mybir.AluOpType.is_ge
    sub = mybir.AluOpType.subtract
    mult = mybir.AluOpType.mult
    add = mybir.AluOpType.add

    with tile.TileContext(nc) as tc:
        with (
            tc.tile_pool(name="const", bufs=1) as const_pool,
            tc.tile_pool(name="xbin_pool", bufs=1) as xbin_pool,
            tc.tile_pool(name="xstage", bufs=3) as xstage_pool,
            tc.tile_pool(name="wstage", bufs=6) as wstage_pool,
            tc.tile_pool(name="wbin", bufs=2) as wbin_pool,
            tc.tile_pool(name="evict", bufs=8) as evict_pool,
            tc.tile_pool(name="psum", bufs=8, space="PSUM") as psum_pool,
        ):
            bias_sb = const_pool.tile([P, o_blocks], mybir.dt.float32, name="bias_sb")
            nc.sync.dma_start(bias_sb[:], biasT[:])

            # Diagnostic-only: pre-filled operand tiles living outside the
            # timed repeat loop.
            xbin_fixed = wbin_fixed = None
            if diag in ("no_x", "no_bin", "pe_only"):
                xbin_fixed = const_pool.tile(
                    [P, k_tiles, t_loc], mm_dtype, name="xbin_fixed"
                )
                for _k in range(k_tiles):
                    nc.any.memset(xbin_fixed[:, _k, :], 0.5)
            if diag in ("no_w", "no_bin", "pe_only"):
                wbin_fixed = const_pool.tile(
                    [P, k_tiles, o_group], mm_dtype, name="wbin_fixed"
                )
                nc.any.memset(wbin_fixed[:], 0.5)

            # keep skipped inputs referenced so walrus accepts the NEFF
            if diag in ("no_x", "pe_only"):
                dummy_x = const_pool.tile([P, 16], mybir.dt.float32, name="dummy_x")
                nc.sync.dma_start(dummy_x[:], xT[:P, :16])
            if diag in ("no_w", "pe_only"):
                dummy_w = const_pool.tile([P, 16], mybir.dt.float32, name="dummy_w")
                nc.sync.dma_start(dummy_w[:], wT[:P, :16])

            import contextlib

            rep_ctx = (
                tc.For_i(0, repeat, 1) if repeat > 1 else contextlib.nullcontext()
            )
            with rep_ctx:
                if xbin_fixed is not None:
                    xbin = xbin_fixed
                else:
                    xbin = xbin_pool.tile([P, k_tiles, t_loc], mm_dtype, name="xbin")
                if diag not in ("no_x", "pe_only"):
                    for k in range(k_tiles):
                        xf = xstage_pool.tile(
                            [P, t_loc], mybir.dt.float32, name="xf"
                        )
                        nc.sync.dma_start(xf[:], xT[ts(k, P), :])
                        if diag != "no_bin":
                            nc.vector.tensor_scalar(
                                xbin[:, k, :], xf[:], 0.0, 0.5, ge, sub
                            )

                for og in range(d_out_loc // o_group):
                    if wbin_fixed is not None:
                        wbin = wbin_fixed
                    else:
                        wbin = wbin_pool.tile(
                            [P, k_tiles, o_group], mm_dtype, name="wbin"
                        )
                    if diag not in ("no_w", "pe_only"):
                        for k in range(k_tiles):
                            wf = wstage_pool.tile(
                                [P, o_group], mybir.dt.float32, name="wf"
                            )
                            nc.sync.dma_start(wf[:], wT[ts(k, P), ts(og, o_group)])
                            if diag != "no_bin":
                                nc.vector.tensor_scalar(
                                    wbin[:, k, :], wf[:], 0.0, 0.5, ge, sub
                                )

                    for oi in range(o_per_g):
                        o = og * o_per_g + oi
                        psums = [
                            psum_pool.tile([P, t_tile], mybir.dt.float32, name="psum")
                            for _ in range(t_blocks)
                        ]
                        for kb in range(kb_blocks):
                            for kp in range(0, kb_tiles, k_step):
                                k = kb * kb_tiles + kp
                                for t in range(t_blocks):
                                    nc.tensor.matmul(
                                        psums[t][:],
                                        wbin[:, k : k + k_step, ts(oi, P)],
                                        xbin[:, k : k + k_step, ts(t, t_tile)],
                                        start=(kb == 0 and kp == 0),
                                        stop=(
                                            kb == kb_blocks - 1
                                            and kp == kb_tiles - k_step
                                        ),
                                        perf_mode=perf,
                                    )
                        for t in range(t_blocks):
                            ob = evict_pool.tile(
                                [P, t_tile], mybir.dt.float32, name="ob"
                            )
                            # out = psum*4 + bias[p]  (exact; bias per-partition)
                            nc.vector.tensor_scalar(
                                ob[:],
                                psums[t][:],
                                4.0,
                                bias_sb[:, o : o + 1],
                                mult,
                                add,
                            )
                            nc.sync.dma_start(outT[ts(o, P), ts(t, t_tile)], ob[:])

    nc.compile()
    return nc


_NC_CACHE: dict = {}

# production sharding: 4-way tokens x 2-way out-features
T_GRP, O_GRP = 4, 2
T_LOC = N_TOK // T_GRP  # 2048
O_LOC = D_OUT // O_GRP  # 2048


def _get_nc(key=("v3",)):
    if key not in _NC_CACHE:
        _NC_CACHE[key] = build_nc_v3(t_loc=T_LOC, d_out_loc=O_LOC)
    return _NC_CACHE[key]


def make_in_maps(x, fp_weight, fp_bias):
    """Host-side sharding (layout only: transpose + slice + replicate)."""
    xT = np.ascontiguousarray(np.asarray(x, dtype=np.float32).T)  # [D_IN, N_TOK]
    wT = np.ascontiguousarray(np.asarray(fp_weight, dtype=np.float32).T)
    bias = np.asarray(fp_bias, dtype=np.float32)
    in_maps = []
    for c in range(N_CORES):
        i, j = divmod(c, O_GRP)
        in_maps.append(
            {
                "xT": np.ascontiguousarray(xT[:, i * T_LOC : (i + 1) * T_LOC]),
                "wT": np.ascontiguousarray(wT[:, j * O_LOC : (j + 1) * O_LOC]),
                "bias": np.ascontiguousarray(
                    np.broadcast_to(
                        bias[None, j * O_LOC : (j + 1) * O_LOC], (P, O_LOC)
                    )
                ),
            }
        )
    return in_maps


def assemble(results) -> np.ndarray:
    out = np.empty((N_TOK, D_OUT), np.float32)
    for c in range(N_CORES):
        i, j = divmod(c, O_GRP)
        out[i * T_LOC : (i + 1) * T_LOC, j * O_LOC : (j + 1) * O_LOC] = results[c][
            "out"
        ]
    return out


def kernel(x: np.ndarray, fp_weight: np.ndarray, fp_bias: np.ndarray) -> np.ndarray:
    assert x.shape == (N_TOK, D_IN) and fp_weight.shape == (D_OUT, D_IN)
    from concourse.bass_utils import run_bass_kernel_spmd

    nc = _get_nc()
    in_maps = make_in_maps(x, fp_weight, fp_bias)
    res = run_bass_kernel_spmd(nc, in_maps, core_ids=list(range(N_CORES)))
    return assemble(res.results)
